# revision 1
# baseline (speedup 1.0000x reference)
"""ChebConv (K=3) spectral graph conv on 8 TRN2 NeuronCores.

Strategy (v3, measured 6.2 ms / rel err 0.0024 on the full problem):
  - 8-way vertex sharding: row v is owned by core v % 8. Every core holds
    the same bf16 gather table xtab [V, 256] = all 4 batches' features
    (64 each) per row, rows permuted so each core's rows are contiguous
    and bin-packed (see below). One descriptor then serves all batches,
    which halves GPSIMD descriptor-generation vs per-batch-pair tables.
  - SpMM y = L @ x: dest rows grouped into 64-row REGIONS; the host
    bin-packs rows so every (region, 32768-row source block) cell has
    <= 128 edges (dma_gather indices are int16). Dest CHUNK = 1024 rows
    = 16 regions = 4 psum banks. Per (chunk, src-block): dma_gather
    (two 1024-index calls; >1024 idxs/call crashes the device) pulls
    source rows token-major into SBUF [128 slots, 16 regions, 256 f].
    A selection matrix sel[s, r] = (rowid[s] == r) * val[s] (2 batched
    DVE broadcast ops; rowid/val streamed from host) turns PE matmuls
    into fused scale+segment-reduce: psum[64 rows, 256 f] += sel^T g,
    accumulating the 6 source blocks in PSUM, then ACT copies each bank
    to SBUF bf16 for a row-major table write. No scatter-add anywhere.
  - x1 tables exchanged via one 8-core AllGather.
  - cheb fold: out = x0(W0-W2) + x1 W1 + (L@x1)(2 W2), y2 used raw.
  - Final matmul PE-transposes 128x128 tiles of x0/y1/y2 into feature-
    major SBUF (transpose-mode dma_gather crashes the device), contracts
    with per-batch [64, 64] weights, adds bias -> out [4, VQ, 64] f32.
Known bottleneck (profile): GPSIMD descriptor generation for the 1152
gather calls is ~81% of runtime; everything else overlaps beneath it.
"""

import os
import numpy as np
import ml_dtypes

from concourse import bacc, bass, mybir, tile
from concourse.masks import make_identity

BF16 = ml_dtypes.bfloat16

# problem constants
V = 196608
NNZ = 1769472
B = 4
P = 64
Q = 64
KK = 3

NCORES = 8
NGROUP = 8        # cores sharing the row space (all 8; 4-batch tables)
FEAT = 256        # 4 batches x 64 features per table row
REG = 64          # rows per region
CHROWS = 1024     # rows per dest chunk (= 16 regions = 4 psum banks)
RPC = CHROWS // REG           # regions per chunk = 32
SRCB = 32768      # source block rows (int16 idx limit)
NSRC = 6          # source blocks: 4*VQ / SRCB
CALL_IDX = RPC * 128          # idxs per dma_gather call = 4096
DROWS = 1024      # rows per phase-D tile group


def _src_layout(vq):
    """Number of source blocks and rows per block for the gather tables."""
    tot = NGROUP * vq
    nsrc = max(6, -(-tot // SRCB))
    while tot % nsrc:
        nsrc += 1
    srcb = tot // nsrc
    assert srcb <= SRCB
    return nsrc, srcb


def _bcast_mid(ap, n):
    return bass.AP(ap.tensor, ap.offset, [ap.ap[0], [0, n], ap.ap[1]])


def _bcast_last(ap, n):
    return bass.AP(ap.tensor, ap.offset, [ap.ap[0], ap.ap[1], [0, n]])


def build_nc(VQ):
    dt = mybir.dt
    NCH = VQ // CHROWS
    NDC = VQ // DROWS
    assert VQ % CHROWS == 0 and VQ % DROWS == 0
    nsrc, srcb = _src_layout(VQ)

    nc = bacc.Bacc(None, num_devices=NCORES, debug=False)

    xtab = nc.declare_dram_parameter("xtab", [NGROUP * VQ, FEAT], dt.bfloat16, isOutput=False)
    x0own = nc.declare_dram_parameter("x0own", [VQ, FEAT], dt.bfloat16, isOutput=False)
    gidx = nc.declare_dram_parameter("gidx", [NCH, nsrc, 128, CALL_IDX // 16], dt.int16, isOutput=False)
    gval = nc.declare_dram_parameter("gval", [NCH, nsrc, 128, RPC], dt.bfloat16, isOutput=False)
    grow = nc.declare_dram_parameter("grow", [NCH, nsrc, 128, RPC], dt.bfloat16, isOutput=False)
    iota64 = nc.declare_dram_parameter("iota64", [128, REG], dt.bfloat16, isOutput=False)
    wmat = nc.declare_dram_parameter("wmat", [KK, 128, 64], dt.bfloat16, isOutput=False)
    bias_rep = nc.declare_dram_parameter("bias_rep", [128, 64], dt.float32, isOutput=False)
    out_ext = nc.declare_dram_parameter("out", [4, VQ, 64], dt.float32, isOutput=True)

    y1 = nc.dram_tensor("y1tab", [VQ, FEAT], dt.bfloat16, kind="Internal")
    y2 = nc.dram_tensor("y2tab", [VQ, FEAT], dt.bfloat16, kind="Internal")
    x1all = nc.dram_tensor("x1all", [NGROUP * VQ, FEAT], dt.bfloat16, kind="Internal")

    groups = [[0, 1, 2, 3, 4, 5, 6, 7]]

    with tile.TileContext(nc) as tc:
        with (
            tc.tile_pool(name="sb", bufs=5) as sb,
            tc.tile_pool(name="ysb", bufs=8) as ysbp,
            tc.tile_pool(name="xt", bufs=2) as xtp,
            tc.tile_pool(name="consts", bufs=1) as consts,
            tc.tile_pool(name="psum", bufs=8, space="PSUM") as pp,
        ):
            iota_t = consts.tile([128, REG], dt.bfloat16, tag="iota")
            nc.sync.dma_start(iota_t[:, :], iota64[:, :])
            w_ts = []
            for t in range(KK):
                w_t = consts.tile([128, 64], dt.bfloat16, tag=f"w{t}")
                nc.sync.dma_start(w_t[:, :], wmat[t, :, :])
                w_ts.append(w_t)
            bias_t = consts.tile([128, 64], dt.float32, tag="bias")
            nc.sync.dma_start(bias_t[:, :], bias_rep[:, :])
            ident_t = consts.tile([128, 128], dt.bfloat16, tag="ident")
            make_identity(nc, ident_t[:, :])
            tc.strict_bb_all_engine_barrier()

            def spmm(src_table, ytab):
                for c in range(NCH):
                    banks = []
                    for _bi in range(4):
                        bank_t = pp.tile([128, 512], dt.float32, tag="ps")
                        banks.append(bank_t)
                    for s in range(nsrc):
                        idx_t = sb.tile([128, CALL_IDX // 16], dt.int16, tag="idx")
                        nc.sync.dma_start(idx_t[:, :], gidx[c, s, :, :])
                        val_t = sb.tile([128, RPC], dt.bfloat16, tag="val")
                        nc.sync.dma_start(val_t[:, :], gval[c, s, :, :])
                        row_t = sb.tile([128, RPC], dt.bfloat16, tag="row")
                        nc.sync.dma_start(row_t[:, :], grow[c, s, :, :])
                        g_t = sb.tile([128, RPC, FEAT], dt.bfloat16, tag="g")
                        # device limit: <=1024 idxs per dma_gather call
                        for q in range(CALL_IDX // 1024):
                            nc.gpsimd.dma_gather(
                                out_ap=g_t[:, 8 * q:8 * (q + 1), :],
                                in_ap=src_table[s * srcb:(s + 1) * srcb, :],
                                idxs_ap=idx_t[:, 64 * q:64 * (q + 1)],
                                num_idxs=1024, num_idxs_reg=1024,
                                elem_size=FEAT,
                            )
                        # sel[slot, r, reg] = (rowid[slot, r] == reg) * val[slot, r]
                        eq_t = sb.tile([128, RPC, REG], dt.bfloat16, tag="eq")
                        nc.vector.tensor_tensor(
                            out=eq_t[:, :, :],
                            in0=_bcast_mid(iota_t[:, :], RPC),
                            in1=_bcast_last(row_t[:, :], REG),
                            op=mybir.AluOpType.is_equal,
                        )
                        sel_t = sb.tile([128, RPC, REG], dt.bfloat16, tag="sel")
                        nc.vector.tensor_tensor(
                            out=sel_t[:, :, :],
                            in0=eq_t[:, :, :],
                            in1=_bcast_last(val_t[:, :], REG),
                            op=mybir.AluOpType.mult,
                        )
                        # region r -> out [64 rows, 256 f] in psum bank r//4
                        # at partition offset 64*(rb%2), free offset 256*(rb//2)
                        for r in range(RPC):
                            pt = banks[r // 4]
                            rb = r % 4
                            po = 64 * (rb % 2)
                            fo = 256 * (rb // 2)
                            nc.tensor.matmul(
                                pt[po:po + 64, fo:fo + 256],
                                lhsT=sel_t[:, r, :],
                                rhs=g_t[:, r, :],
                                start=(s == 0 and rb // 2 == 0),
                                stop=(s == nsrc - 1 and rb // 2 == 1),
                                skip_group_check=True,
                            )
                    for bk in range(4):
                        ysb = ysbp.tile([128, 512], dt.bfloat16, tag="ysb")
                        nc.scalar.copy(ysb[:, :], banks[bk][:, :])
                        # dest row 128*a + 64*b + j <- ysb[64*b + j, a, :]
                        dst = ytab[c * CHROWS + bk * 256:
                                   c * CHROWS + (bk + 1) * 256, :]
                        nc.sync.dma_start(
                            dst.rearrange("(a b j) f -> (b j) a f", a=2, b=2),
                            ysb[:, :].rearrange("p (a f) -> p a f", a=2))

                tc.strict_bb_all_engine_barrier()

            ph = os.environ.get("KPHASES", "1234")
            # ---------------- phase A ----------------
            if "1" in ph:
                spmm(xtab, y1)

            # ---------------- phase B ----------------
            if "2" in ph:
                nc.gpsimd.collective_compute(
                    "AllGather", mybir.AluOpType.bypass,
                    replica_groups=groups,
                    ins=[y1[:, :]], outs=[x1all[:, :]],
                )
                tc.strict_bb_all_engine_barrier()

            # ---------------- phase C ----------------
            if "3" in ph:
                spmm(x1all, y2)

            # ---------------- phase D ----------------
            for c in (range(NDC) if "4" in ph else []):
                xts = []
                for nm, tbl in (("x0T", x0own), ("y1T", y1), ("y2T", y2)):
                    xr = xtp.tile([128, 8, FEAT], dt.bfloat16, tag=nm + "r")
                    nc.sync.dma_start(
                        xr[:, :, :],
                        tbl[c * DROWS:(c + 1) * DROWS, :].rearrange(
                            "(a p) f -> p a f", p=128))
                    xt2 = []
                    for half in range(2):
                        xt = xtp.tile([128, 8, 128], dt.bfloat16,
                                      tag=f"{nm}{half}")
                        for a8 in range(8):
                            ptt = pp.tile([128, 128], dt.bfloat16, tag="ps")
                            nc.tensor.transpose(
                                out=ptt[:, :],
                                in_=xr[:, a8, 128 * half:128 * (half + 1)],
                                identity=ident_t[:, :])
                            nc.scalar.copy(xt[:, a8, :], ptt[:, :])
                        xt2.append(xt)
                    xts.append(xt2)
                for b in range(4):
                    pt = pp.tile([128, 512], dt.float32, tag="ps")
                    for j in range(8):
                        for t in range(KK):
                            nc.tensor.matmul(
                                pt[:, 64 * j:64 * (j + 1)],
                                lhsT=xts[t][b // 2][64 * (b % 2):64 * (b % 2 + 1), j, :],
                                rhs=w_ts[t][64 * (b % 2):64 * (b % 2 + 1), :],
                                start=(t == 0 and j == 0),
                                stop=(t == KK - 1 and j == 7),
                                skip_group_check=True,
                            )
                    osb = sb.tile([128, 8, 64], dt.float32, tag="osbD")
                    pt3 = bass.AP(pt[:, :].tensor, pt[:, :].offset,
                                  [pt[:, :].ap[0], [64, 8], [1, 64]])
                    nc.vector.tensor_tensor(
                        out=osb[:, :, :], in0=pt3,
                        in1=_bcast_mid(bias_t[:, :], 8),
                        op=mybir.AluOpType.add,
                    )
                    dst = out_ext[b, c * DROWS:(c + 1) * DROWS, :].rearrange(
                        "(j p) q -> p j q", p=128)
                    nc.sync.dma_start(dst, osb[:, :, :])

    nc.finalize()
    return nc


# --------------------------------------------------------------------------
# host-side preparation
# --------------------------------------------------------------------------

def _repair_pack(region_of, deg, rng):
    """Repair an assignment of rows to 64-row regions so that every
    (region, src) cell <= 128. In-place swaps; strict-progress accepted."""
    vq, nsrc = deg.shape
    nreg = vq // REG
    for _ in range(2000):
        cells = np.zeros((nreg, nsrc), np.int64)
        for s in range(nsrc):
            cells[:, s] = np.bincount(region_of, weights=deg[:, s],
                                      minlength=nreg)
        over = np.argwhere(cells > 128)
        if len(over) == 0:
            return region_of
        r, s = over[0]
        rows_r = np.where(region_of == r)[0]
        cand_a = rows_r[np.argsort(deg[rows_r, s])[::-1][:16]]
        order = np.argsort(cells[:, s])
        done = False
        for a in cand_a:
            for t in order[:256]:
                if t == r:
                    continue
                rows_t = np.where(region_of == t)[0]
                cand_b = rows_t[np.argsort(deg[rows_t, s])[:8]]
                for b in cand_b:
                    new_r = cells[r] - deg[a] + deg[b]
                    new_t = cells[t] + deg[a] - deg[b]
                    others = np.delete(new_r, s)
                    if (new_r[s] < cells[r, s] and (others <= 128).all()
                            and (new_t <= 128).all()):
                        region_of[a], region_of[b] = t, r
                        done = True
                        break
                if done:
                    break
            if done:
                break
        if not done:
            raise RuntimeError("packing repair failed")
    raise RuntimeError("packing did not converge")


def prepare_inputs(lap_vals, x, weight, bias, lap_rows, lap_cols):
    vq = V // NGROUP
    nch = vq // CHROWS
    ndc = vq // DROWS
    nsrc, srcb = _src_layout(vq)

    rows = np.asarray(lap_rows).astype(np.int64)
    cols = np.asarray(lap_cols).astype(np.int64)
    vals = np.asarray(lap_vals).astype(np.float32)
    x = np.asarray(x)
    weight = np.asarray(weight)
    bias = np.asarray(bias)

    rng = np.random.default_rng(12345)
    v_all = np.arange(V)
    owner = rows % NGROUP
    row_id = rows // NGROUP             # row id within owner core
    e_of = [np.where(owner == h)[0] for h in range(NGROUP)]
    vids_of = [np.where(v_all % NGROUP == h)[0] for h in range(NGROUP)]

    def region_to_pos(region_of):
        """positions: rows of a region get slots 0..REG-1 (stable order)."""
        srt = np.argsort(region_of, kind="stable")
        pos = np.empty(vq, np.int64)
        pos[srt] = np.arange(vq)
        return pos                      # = region*REG + slot

    # iterate packing <-> block assignment to a fixed point (in-place repair)
    region_core = [None] * NGROUP
    pos_core = [rng.permutation(vq) for _ in range(NGROUP)]
    for h in range(NGROUP):
        perm = rng.permutation(vq)
        ro = np.empty(vq, np.int64)
        ro[perm] = np.arange(vq) // REG
        region_core[h] = ro
        pos_core[h] = region_to_pos(ro)
    for attempt in range(12):
        pos_of = np.empty(V, np.int64)
        for h in range(NGROUP):
            pos_of[vids_of[h]] = pos_core[h][v_all[vids_of[h]] // NGROUP]
        tabpos = (v_all % NGROUP) * vq + pos_of
        col_blk_all = tabpos[cols] // srcb
        all_ok = True
        for h in range(NGROUP):
            e_h = e_of[h]
            deg = np.zeros((vq, nsrc), np.int64)
            np.add.at(deg, (row_id[e_h], col_blk_all[e_h]), 1)
            cells = np.zeros((vq // REG, nsrc), np.int64)
            for s in range(nsrc):
                cells[:, s] = np.bincount(region_core[h], weights=deg[:, s],
                                          minlength=vq // REG)
            if (cells > 128).any():
                all_ok = False
                _repair_pack(region_core[h], deg, rng)
                pos_core[h] = region_to_pos(region_core[h])
        if all_ok:
            break
    else:
        raise RuntimeError("packing/block iteration did not converge")

    pos_of = np.empty(V, np.int64)
    for h in range(NGROUP):
        pos_of[vids_of[h]] = pos_core[h][v_all[vids_of[h]] // NGROUP]
    tabpos = (v_all % NGROUP) * vq + pos_of
    col_tab = tabpos[cols]
    col_blk = col_tab // srcb

    # --- build streams per core
    col_loc = (col_tab % srcb).astype(np.int16)
    gidx_c, gval_c, grow_c = [], [], []
    for h in range(NGROUP):
        e_h = e_of[h]
        rpos = pos_of[rows[e_h]]
        reg = rpos // REG
        slot = rpos % REG
        blk = col_blk[e_h]
        ch = reg // RPC
        rl = reg % RPC
        # order edges by (ch, blk, rl) then arbitrary; position within cell:
        key = (ch * nsrc + blk) * RPC + rl
        order = np.argsort(key, kind="stable")
        ks = key[order]
        starts = np.searchsorted(ks, np.arange(nch * nsrc * RPC))
        counts = np.diff(np.concatenate([starts, [len(ks)]]))
        assert counts.max() <= 128, f"cell overflow {counts.max()}"
        within = np.arange(len(ks)) - starts[ks]
        # token index within call: rl*128 + within ; call = (ch, blk)
        gidx = np.zeros((nch, nsrc, CALL_IDX), np.int16)
        gval = np.zeros((nch, nsrc, RPC, 128), np.float32)
        grow = np.full((nch, nsrc, RPC, 128), 255.0, np.float32)
        eo = e_h[order]
        ch_o, blk_o, rl_o = ch[order], blk[order], rl[order]
        tok = rl_o * 128 + within
        gidx[ch_o, blk_o, tok] = col_loc[e_h][order]
        gval[ch_o, blk_o, rl_o, within] = vals[eo]
        grow[ch_o, blk_o, rl_o, within] = slot[order]
        # wrap idx: [NCH, nsrc, 128, CALL_IDX//16], idx i -> [i%16, i//16]
        gw = gidx.reshape(nch, nsrc, CALL_IDX // 16, 16).transpose(0, 1, 3, 2)
        gw = np.broadcast_to(gw[:, :, None, :, :],
                             (nch, nsrc, 8, 16, CALL_IDX // 16)
                             ).reshape(nch, nsrc, 128, CALL_IDX // 16)
        gidx_c.append(np.ascontiguousarray(gw))
        gval_c.append(gval.transpose(0, 1, 3, 2).astype(BF16).copy())
        grow_c.append(grow.transpose(0, 1, 3, 2).astype(BF16).copy())

    # --- table: all 4 batches, 256 feats per row
    feat = np.concatenate([x[0], x[1], x[2], x[3]], axis=1).astype(BF16)
    xtab = np.zeros((NGROUP * vq, FEAT), BF16)
    xtab[tabpos] = feat


    iota64 = np.broadcast_to(np.arange(REG, dtype=np.float32)[None, :],
                             (128, REG)).astype(BF16).copy()

    wm = weight.reshape(KK * P, Q)
    wk = wm.reshape(P, KK, Q).transpose(1, 0, 2)
    wfix = np.stack([wk[0] - wk[2], wk[1], 2.0 * wk[2]])
    wfix = np.concatenate([wfix, wfix], axis=1).astype(BF16)

    bias_rep = np.tile(np.asarray(bias, np.float32)[None, :], (128, 1))

    in_maps = []
    for core in range(NCORES):
        h = core
        in_maps.append({
            "xtab": xtab,
            "x0own": xtab[h * vq:(h + 1) * vq],
            "gidx": gidx_c[h], "gval": gval_c[h], "grow": grow_c[h],
            "iota64": iota64,
            "wmat": wfix, "bias_rep": bias_rep,
        })
    return in_maps, vq, pos_of


def assemble_output(results, vq, pos_of):
    out = np.zeros((B, V, Q), np.float32)
    for core in range(NCORES):
        co = results[core]["out"]
        v_ids = np.where(np.arange(V) % NGROUP == core)[0]
        for b in range(B):
            out[b, v_ids] = co[b][pos_of[v_ids]]
    return out


_NC_CACHE = {}


def kernel(lap_vals, x, weight, bias, lap_rows, lap_cols):
    from concourse.bass_utils import run_bass_kernel_spmd

    in_maps, vq, pos_of = prepare_inputs(
        lap_vals, x, weight, bias, lap_rows, lap_cols)

    if vq not in _NC_CACHE:
        _NC_CACHE[vq] = build_nc(vq)
    nc = _NC_CACHE[vq]

    res = run_bass_kernel_spmd(nc, in_maps, core_ids=list(range(NCORES)))
    return assemble_output(res.results, vq, pos_of)



# revision 8
# speedup vs baseline: 1.0278x; 1.0278x over previous
"""ChebConv (K=3) spectral graph conv on 8 TRN2 NeuronCores.

Strategy (v3, measured 6.2 ms / rel err 0.0024 on the full problem):
  - 8-way vertex sharding: row v is owned by core v % 8. Every core holds
    the same bf16 gather table xtab [V, 256] = all 4 batches' features
    (64 each) per row, rows permuted so each core's rows are contiguous
    and bin-packed (see below). One descriptor then serves all batches,
    which halves GPSIMD descriptor-generation vs per-batch-pair tables.
  - SpMM y = L @ x: dest rows grouped into 64-row REGIONS; the host
    bin-packs rows so every (region, 32768-row source block) cell has
    <= 128 edges (dma_gather indices are int16). Dest CHUNK = 1024 rows
    = 16 regions = 4 psum banks. Per (chunk, src-block): dma_gather
    (two 1024-index calls; >1024 idxs/call crashes the device) pulls
    source rows token-major into SBUF [128 slots, 16 regions, 256 f].
    A selection matrix sel[s, r] = (rowid[s] == r) * val[s] (2 batched
    DVE broadcast ops; rowid/val streamed from host) turns PE matmuls
    into fused scale+segment-reduce: psum[64 rows, 256 f] += sel^T g,
    accumulating the 6 source blocks in PSUM, then ACT copies each bank
    to SBUF bf16 for a row-major table write. No scatter-add anywhere.
  - x1 tables exchanged via one 8-core AllGather.
  - cheb fold: out = x0(W0-W2) + x1 W1 + (L@x1)(2 W2), y2 used raw.
  - Final matmul PE-transposes 128x128 tiles of x0/y1/y2 into feature-
    major SBUF (transpose-mode dma_gather crashes the device), contracts
    with per-batch [64, 64] weights, adds bias -> out [4, VQ, 64] f32.
Known bottleneck (profile): GPSIMD descriptor generation for the 1152
gather calls is ~81% of runtime; everything else overlaps beneath it.
"""

import os
import numpy as np
import ml_dtypes

from concourse import bacc, bass, mybir, tile
from concourse.masks import make_identity

BF16 = ml_dtypes.bfloat16

# problem constants
V = 196608
NNZ = 1769472
B = 4
P = 64
Q = 64
KK = 3

NCORES = 8
NGROUP = 8        # cores sharing the row space (all 8; 4-batch tables)
FEAT = 256        # 4 batches x 64 features per table row
REG = 64          # rows per region
CHROWS = 1024     # rows per dest chunk (= 16 regions = 4 psum banks)
RPC = CHROWS // REG           # regions per chunk = 32
SRCB = 32768      # source block rows (int16 idx limit)
NSRC = 6          # source blocks: 4*VQ / SRCB
CALL_IDX = RPC * 128          # idxs per dma_gather call = 4096
DROWS = 1024      # rows per phase-D tile group


def _src_layout(vq):
    """Number of source blocks and rows per block for the gather tables."""
    tot = NGROUP * vq
    nsrc = max(6, -(-tot // SRCB))
    while tot % nsrc:
        nsrc += 1
    srcb = tot // nsrc
    assert srcb <= SRCB
    return nsrc, srcb


def _bcast_mid(ap, n):
    return bass.AP(ap.tensor, ap.offset, [ap.ap[0], [0, n], ap.ap[1]])


def _bcast_last(ap, n):
    return bass.AP(ap.tensor, ap.offset, [ap.ap[0], ap.ap[1], [0, n]])


def build_nc(VQ):
    dt = mybir.dt
    NCH = VQ // CHROWS
    NDC = VQ // DROWS
    assert VQ % CHROWS == 0 and VQ % DROWS == 0
    nsrc, srcb = _src_layout(VQ)

    nc = bacc.Bacc(None, num_devices=NCORES, debug=False, num_swdge_queues=4)

    xtab = nc.declare_dram_parameter("xtab", [NGROUP * VQ, FEAT], dt.bfloat16, isOutput=False)
    x0own = nc.declare_dram_parameter("x0own", [VQ, FEAT], dt.bfloat16, isOutput=False)
    gidx = nc.declare_dram_parameter("gidx", [NCH, nsrc, 128, CALL_IDX // 16], dt.int16, isOutput=False)
    gval = nc.declare_dram_parameter("gval", [NCH, nsrc, 128, RPC], dt.bfloat16, isOutput=False)
    grow = nc.declare_dram_parameter("grow", [NCH, nsrc, 128, RPC], dt.bfloat16, isOutput=False)
    iota64 = nc.declare_dram_parameter("iota64", [128, REG], dt.bfloat16, isOutput=False)
    wmat = nc.declare_dram_parameter("wmat", [KK, 128, 64], dt.bfloat16, isOutput=False)
    bias_rep = nc.declare_dram_parameter("bias_rep", [128, 64], dt.float32, isOutput=False)
    out_ext = nc.declare_dram_parameter("out", [4, VQ, 64], dt.float32, isOutput=True)

    y1 = nc.dram_tensor("y1tab", [VQ, FEAT], dt.bfloat16, kind="Internal")
    y2 = nc.dram_tensor("y2tab", [VQ, FEAT], dt.bfloat16, kind="Internal")
    x1all = nc.dram_tensor("x1all", [NGROUP * VQ, FEAT], dt.bfloat16,
                           kind="Internal", addr_space="Shared")

    groups = [[0, 1, 2, 3, 4, 5, 6, 7]]

    with tile.TileContext(nc) as tc:
        with (
            tc.tile_pool(name="sb", bufs=5) as sb,
            tc.tile_pool(name="ysb", bufs=8) as ysbp,
            tc.tile_pool(name="xt", bufs=2) as xtp,
            tc.tile_pool(name="consts", bufs=1) as consts,
            tc.tile_pool(name="psum", bufs=8, space="PSUM") as pp,
        ):
            iota_t = consts.tile([128, REG], dt.bfloat16, tag="iota")
            nc.sync.dma_start(iota_t[:, :], iota64[:, :])
            w_ts = []
            for t in range(KK):
                w_t = consts.tile([128, 64], dt.bfloat16, tag=f"w{t}")
                nc.sync.dma_start(w_t[:, :], wmat[t, :, :])
                w_ts.append(w_t)
            bias_t = consts.tile([128, 64], dt.float32, tag="bias")
            nc.sync.dma_start(bias_t[:, :], bias_rep[:, :])
            ident_t = consts.tile([128, 128], dt.bfloat16, tag="ident")
            make_identity(nc, ident_t[:, :])
            tc.strict_bb_all_engine_barrier()

            qctr = [0]

            def spmm(src_table, ytab, epilogue=None):
                for c in range(NCH):
                    banks = []
                    for _bi in range(4):
                        bank_t = pp.tile([128, 512], dt.float32, tag="ps")
                        banks.append(bank_t)
                    for s in range(nsrc):
                        idx_t = sb.tile([128, CALL_IDX // 16], dt.int16, tag="idx")
                        nc.sync.dma_start(idx_t[:, :], gidx[c, s, :, :])
                        val_t = sb.tile([128, RPC], dt.bfloat16, tag="val")
                        nc.sync.dma_start(val_t[:, :], gval[c, s, :, :])
                        row_t = sb.tile([128, RPC], dt.bfloat16, tag="row")
                        nc.sync.dma_start(row_t[:, :], grow[c, s, :, :])
                        g_t = sb.tile([128, RPC, FEAT], dt.bfloat16, tag="g")
                        # device limit: <=1024 idxs per dma_gather call
                        for q in range(CALL_IDX // 1024):
                            nc.gpsimd.dma_gather(
                                out_ap=g_t[:, 8 * q:8 * (q + 1), :],
                                in_ap=src_table[s * srcb:(s + 1) * srcb, :],
                                idxs_ap=idx_t[:, 64 * q:64 * (q + 1)],
                                num_idxs=1024, num_idxs_reg=1024,
                                elem_size=FEAT,
                                queue_num=qctr[0] % 4,
                            )
                            qctr[0] += 1
                        # sel[slot, r, reg] = (rowid[slot, r] == reg) * val[slot, r]
                        eq_t = sb.tile([128, RPC, REG], dt.bfloat16, tag="eq")
                        nc.vector.tensor_tensor(
                            out=eq_t[:, :, :],
                            in0=_bcast_mid(iota_t[:, :], RPC),
                            in1=_bcast_last(row_t[:, :], REG),
                            op=mybir.AluOpType.is_equal,
                        )
                        sel_t = sb.tile([128, RPC, REG], dt.bfloat16, tag="sel")
                        nc.vector.tensor_tensor(
                            out=sel_t[:, :, :],
                            in0=eq_t[:, :, :],
                            in1=_bcast_last(val_t[:, :], REG),
                            op=mybir.AluOpType.mult,
                        )
                        # region r -> out [64 rows, 256 f] in psum bank r//4
                        # at partition offset 64*(rb%2), free offset 256*(rb//2)
                        for r in range(RPC):
                            pt = banks[r // 4]
                            rb = r % 4
                            po = 64 * (rb % 2)
                            fo = 256 * (rb // 2)
                            nc.tensor.matmul(
                                pt[po:po + 64, fo:fo + 256],
                                lhsT=sel_t[:, r, :],
                                rhs=g_t[:, r, :],
                                start=(s == 0 and rb // 2 == 0),
                                stop=(s == nsrc - 1 and rb // 2 == 1),
                                skip_group_check=True,
                            )
                    for bk in range(4):
                        ysb = ysbp.tile([128, 512], dt.bfloat16, tag="ysb")
                        nc.scalar.copy(ysb[:, :], banks[bk][:, :])
                        # dest row 128*a + 64*b + j <- ysb[64*b + j, a, :]
                        dst = ytab[c * CHROWS + bk * 256:
                                   c * CHROWS + (bk + 1) * 256, :]
                        nc.sync.dma_start(
                            dst.rearrange("(a b j) f -> (b j) a f", a=2, b=2),
                            ysb[:, :].rearrange("p (a f) -> p a f", a=2))
                    if epilogue is not None:
                        epilogue(c)

                tc.strict_bb_all_engine_barrier()

            def phase_d_chunk(c):
                xts = []
                for nm, tbl in (("x0T", x0own), ("y1T", y1), ("y2T", y2)):
                    xr = xtp.tile([128, 8, FEAT], dt.bfloat16, tag=nm + "r")
                    nc.sync.dma_start(
                        xr[:, :, :],
                        tbl[c * DROWS:(c + 1) * DROWS, :].rearrange(
                            "(a p) f -> p a f", p=128))
                    xt2 = []
                    for half in range(2):
                        xt = xtp.tile([128, 8, 128], dt.bfloat16,
                                      tag=f"{nm}{half}")
                        for a8 in range(8):
                            ptt = pp.tile([128, 128], dt.bfloat16, tag="ps")
                            nc.tensor.transpose(
                                out=ptt[:, :],
                                in_=xr[:, a8, 128 * half:128 * (half + 1)],
                                identity=ident_t[:, :])
                            nc.scalar.copy(xt[:, a8, :], ptt[:, :])
                        xt2.append(xt)
                    xts.append(xt2)
                for b in range(4):
                    pt = pp.tile([128, 512], dt.float32, tag="ps")
                    for j in range(8):
                        for t in range(KK):
                            nc.tensor.matmul(
                                pt[:, 64 * j:64 * (j + 1)],
                                lhsT=xts[t][b // 2][64 * (b % 2):64 * (b % 2 + 1), j, :],
                                rhs=w_ts[t][64 * (b % 2):64 * (b % 2 + 1), :],
                                start=(t == 0 and j == 0),
                                stop=(t == KK - 1 and j == 7),
                                skip_group_check=True,
                            )
                    osb = sb.tile([128, 8, 64], dt.float32, tag="osbD")
                    pt3 = bass.AP(pt[:, :].tensor, pt[:, :].offset,
                                  [pt[:, :].ap[0], [64, 8], [1, 64]])
                    nc.vector.tensor_tensor(
                        out=osb[:, :, :], in0=pt3,
                        in1=_bcast_mid(bias_t[:, :], 8),
                        op=mybir.AluOpType.add,
                    )
                    dst = out_ext[b, c * DROWS:(c + 1) * DROWS, :].rearrange(
                        "(j p) q -> p j q", p=128)
                    nc.sync.dma_start(dst, osb[:, :, :])

            ph = os.environ.get("KPHASES", "1234")
            # ---------------- phase A ----------------
            if "1" in ph:
                spmm(xtab, y1)

            # ---------------- phase B ----------------
            if "2" in ph:
                nc.gpsimd.collective_compute(
                    "AllGather", mybir.AluOpType.bypass,
                    replica_groups=groups,
                    ins=[y1[:, :]], outs=[x1all[:, :]],
                )
                tc.strict_bb_all_engine_barrier()

            # ---------------- phase C + D interleaved per chunk ------------
            if "3" in ph:
                spmm(x1all, y2,
                     epilogue=(phase_d_chunk if "4" in ph else None))

    nc.finalize()
    return nc


# --------------------------------------------------------------------------
# host-side preparation
# --------------------------------------------------------------------------

def _repair_pack(region_of, deg, rng):
    """Repair an assignment of rows to 64-row regions so that every
    (region, src) cell <= 128. In-place swaps; strict-progress accepted."""
    vq, nsrc = deg.shape
    nreg = vq // REG
    for _ in range(2000):
        cells = np.zeros((nreg, nsrc), np.int64)
        for s in range(nsrc):
            cells[:, s] = np.bincount(region_of, weights=deg[:, s],
                                      minlength=nreg)
        over = np.argwhere(cells > 128)
        if len(over) == 0:
            return region_of
        r, s = over[0]
        rows_r = np.where(region_of == r)[0]
        cand_a = rows_r[np.argsort(deg[rows_r, s])[::-1][:16]]
        order = np.argsort(cells[:, s])
        done = False
        for a in cand_a:
            for t in order[:256]:
                if t == r:
                    continue
                rows_t = np.where(region_of == t)[0]
                cand_b = rows_t[np.argsort(deg[rows_t, s])[:8]]
                for b in cand_b:
                    new_r = cells[r] - deg[a] + deg[b]
                    new_t = cells[t] + deg[a] - deg[b]
                    others = np.delete(new_r, s)
                    if (new_r[s] < cells[r, s] and (others <= 128).all()
                            and (new_t <= 128).all()):
                        region_of[a], region_of[b] = t, r
                        done = True
                        break
                if done:
                    break
            if done:
                break
        if not done:
            raise RuntimeError("packing repair failed")
    raise RuntimeError("packing did not converge")


def prepare_inputs(lap_vals, x, weight, bias, lap_rows, lap_cols):
    vq = V // NGROUP
    nch = vq // CHROWS
    ndc = vq // DROWS
    nsrc, srcb = _src_layout(vq)

    rows = np.asarray(lap_rows).astype(np.int64)
    cols = np.asarray(lap_cols).astype(np.int64)
    vals = np.asarray(lap_vals).astype(np.float32)
    x = np.asarray(x)
    weight = np.asarray(weight)
    bias = np.asarray(bias)

    rng = np.random.default_rng(12345)
    v_all = np.arange(V)
    owner = rows % NGROUP
    row_id = rows // NGROUP             # row id within owner core
    e_of = [np.where(owner == h)[0] for h in range(NGROUP)]
    vids_of = [np.where(v_all % NGROUP == h)[0] for h in range(NGROUP)]

    def region_to_pos(region_of):
        """positions: rows of a region get slots 0..REG-1 (stable order)."""
        srt = np.argsort(region_of, kind="stable")
        pos = np.empty(vq, np.int64)
        pos[srt] = np.arange(vq)
        return pos                      # = region*REG + slot

    # iterate packing <-> block assignment to a fixed point (in-place repair)
    region_core = [None] * NGROUP
    pos_core = [rng.permutation(vq) for _ in range(NGROUP)]
    for h in range(NGROUP):
        perm = rng.permutation(vq)
        ro = np.empty(vq, np.int64)
        ro[perm] = np.arange(vq) // REG
        region_core[h] = ro
        pos_core[h] = region_to_pos(ro)
    for attempt in range(12):
        pos_of = np.empty(V, np.int64)
        for h in range(NGROUP):
            pos_of[vids_of[h]] = pos_core[h][v_all[vids_of[h]] // NGROUP]
        tabpos = (v_all % NGROUP) * vq + pos_of
        col_blk_all = tabpos[cols] // srcb
        all_ok = True
        for h in range(NGROUP):
            e_h = e_of[h]
            deg = np.zeros((vq, nsrc), np.int64)
            np.add.at(deg, (row_id[e_h], col_blk_all[e_h]), 1)
            cells = np.zeros((vq // REG, nsrc), np.int64)
            for s in range(nsrc):
                cells[:, s] = np.bincount(region_core[h], weights=deg[:, s],
                                          minlength=vq // REG)
            if (cells > 128).any():
                all_ok = False
                _repair_pack(region_core[h], deg, rng)
                pos_core[h] = region_to_pos(region_core[h])
        if all_ok:
            break
    else:
        raise RuntimeError("packing/block iteration did not converge")

    pos_of = np.empty(V, np.int64)
    for h in range(NGROUP):
        pos_of[vids_of[h]] = pos_core[h][v_all[vids_of[h]] // NGROUP]
    tabpos = (v_all % NGROUP) * vq + pos_of
    col_tab = tabpos[cols]
    col_blk = col_tab // srcb

    # --- build streams per core
    col_loc = (col_tab % srcb).astype(np.int16)
    gidx_c, gval_c, grow_c = [], [], []
    for h in range(NGROUP):
        e_h = e_of[h]
        rpos = pos_of[rows[e_h]]
        reg = rpos // REG
        slot = rpos % REG
        blk = col_blk[e_h]
        ch = reg // RPC
        rl = reg % RPC
        # order edges by (ch, blk, rl) then arbitrary; position within cell:
        key = (ch * nsrc + blk) * RPC + rl
        order = np.argsort(key, kind="stable")
        ks = key[order]
        starts = np.searchsorted(ks, np.arange(nch * nsrc * RPC))
        counts = np.diff(np.concatenate([starts, [len(ks)]]))
        assert counts.max() <= 128, f"cell overflow {counts.max()}"
        within = np.arange(len(ks)) - starts[ks]
        # token index within call: rl*128 + within ; call = (ch, blk)
        gidx = np.zeros((nch, nsrc, CALL_IDX), np.int16)
        gval = np.zeros((nch, nsrc, RPC, 128), np.float32)
        grow = np.full((nch, nsrc, RPC, 128), 255.0, np.float32)
        eo = e_h[order]
        ch_o, blk_o, rl_o = ch[order], blk[order], rl[order]
        tok = rl_o * 128 + within
        gidx[ch_o, blk_o, tok] = col_loc[e_h][order]
        gval[ch_o, blk_o, rl_o, within] = vals[eo]
        grow[ch_o, blk_o, rl_o, within] = slot[order]
        # wrap idx: [NCH, nsrc, 128, CALL_IDX//16], idx i -> [i%16, i//16]
        gw = gidx.reshape(nch, nsrc, CALL_IDX // 16, 16).transpose(0, 1, 3, 2)
        gw = np.broadcast_to(gw[:, :, None, :, :],
                             (nch, nsrc, 8, 16, CALL_IDX // 16)
                             ).reshape(nch, nsrc, 128, CALL_IDX // 16)
        gidx_c.append(np.ascontiguousarray(gw))
        gval_c.append(gval.transpose(0, 1, 3, 2).astype(BF16).copy())
        grow_c.append(grow.transpose(0, 1, 3, 2).astype(BF16).copy())

    # --- table: all 4 batches, 256 feats per row
    feat = np.concatenate([x[0], x[1], x[2], x[3]], axis=1).astype(BF16)
    xtab = np.zeros((NGROUP * vq, FEAT), BF16)
    xtab[tabpos] = feat


    iota64 = np.broadcast_to(np.arange(REG, dtype=np.float32)[None, :],
                             (128, REG)).astype(BF16).copy()

    wm = weight.reshape(KK * P, Q)
    wk = wm.reshape(P, KK, Q).transpose(1, 0, 2)
    wfix = np.stack([wk[0] - wk[2], wk[1], 2.0 * wk[2]])
    wfix = np.concatenate([wfix, wfix], axis=1).astype(BF16)

    bias_rep = np.tile(np.asarray(bias, np.float32)[None, :], (128, 1))

    in_maps = []
    for core in range(NCORES):
        h = core
        in_maps.append({
            "xtab": xtab,
            "x0own": xtab[h * vq:(h + 1) * vq],
            "gidx": gidx_c[h], "gval": gval_c[h], "grow": grow_c[h],
            "iota64": iota64,
            "wmat": wfix, "bias_rep": bias_rep,
        })
    return in_maps, vq, pos_of


def assemble_output(results, vq, pos_of):
    out = np.zeros((B, V, Q), np.float32)
    for core in range(NCORES):
        co = results[core]["out"]
        v_ids = np.where(np.arange(V) % NGROUP == core)[0]
        for b in range(B):
            out[b, v_ids] = co[b][pos_of[v_ids]]
    return out


_NC_CACHE = {}


def kernel(lap_vals, x, weight, bias, lap_rows, lap_cols):
    from concourse.bass_utils import run_bass_kernel_spmd

    in_maps, vq, pos_of = prepare_inputs(
        lap_vals, x, weight, bias, lap_rows, lap_cols)

    if vq not in _NC_CACHE:
        _NC_CACHE[vq] = build_nc(vq)
    nc = _NC_CACHE[vq]

    res = run_bass_kernel_spmd(nc, in_maps, core_ids=list(range(NCORES)))
    return assemble_output(res.results, vq, pos_of)



# revision 13
# speedup vs baseline: 2.3836x; 2.3190x over previous
"""ChebConv (K=3) spectral graph conv on 8 TRN2 NeuronCores — v5.

Measured 2.62 ms / rel err 0.0024 on the full problem (baseline 6.74 ms).

Design (vs the v3 baseline: dense cells, stall-free desc-gen, fused tail):
  - 8-way vertex sharding, identity row permutation (row v owned by core
    v % 8 at slot v // 8). Gather table [V, 256] bf16 = 4 batches x 64
    feats; table layout is PIECE-major (4 pieces per core's row range) so
    the x1 AllGather can be issued in 4 pipelined slices.
  - SpMM y = L @ x per dest CHUNK of 1024 rows (4 psum banks; 8 GROUPS of
    128 rows; group g -> bank g//2, feat-offset (g%2)*256). Per (chunk,
    32768-row src block) cell: tokens (edges) sorted by dest group, packed
    DENSELY; CAP = round128(max-over-cores count) indices (pad idx 0 /
    val 0) gathered token-major into SBUF [128, CAP/128, 256] in <=1024-idx
    dma_gather calls (>1024 crashes the device). Calls rotate over the 4
    SWDGE queues so descriptor-ring backpressure never stalls GPSIMD
    desc-gen (the v3 bottleneck: 5.05 ms GPSIMD busy -> 1.8 ms).
  - Reduction: per (tile j, group g) PAIR (union across cores => identical
    static structure on all 8 SPMD cores): sel[p, 128] = (rowlo[p, pair]
    == iota128) * val[p, pair], two batched DVE ops over [128, NP, 128];
    PE matmul psum[g][128, 256] += sel^T @ g[:, j, :] accumulating across
    blocks. start/stop flags are per BANK (start zeroes the whole 2KB psum
    row on the addressed partitions, so only the bank's first matmul may
    set it).
  - x1 AllGather into a Shared scratchpad tensor, split into 4 pieces,
    each issued one chunk after its y1 rows are written -> overlaps spmm1.
  - cheb fold out = x0(W0-W2) + x1 W1 + (L x1)(2 W2), interleaved per
    chunk into the second SpMM: x0 arrives host-pretransposed (no PE
    transposes), y1 is re-read + PE-transposed, and y2 never touches DRAM
    (its psum->SBUF copies feed the PE transposes directly).
"""

import os
import numpy as np
import ml_dtypes

from concourse import bacc, bass, mybir, tile
from concourse.masks import make_identity

BF16 = ml_dtypes.bfloat16

V = 196608
NNZ = 1769472
B = 4
P = 64
Q = 64
KK = 3

NCORES = 8
NGROUP = 8
FEAT = 256
CHROWS = 1024     # rows per dest chunk (= 4 psum banks)
GR = 128          # rows per group (psum rect)
NGRP = CHROWS // GR
SRCB = 32768
DROWS = 1024
NSPLIT = 4        # allgather pieces, pipelined into spmm1
SINGLE_PACKET = os.environ.get("KSP", "1") != "0"
MAXCALL = int(os.environ.get("KMAXCALL", "1024"))  # idxs per dma_gather call


def _src_layout(vq):
    tot = NGROUP * vq
    nsrc = max(6, -(-tot // SRCB))
    while tot % nsrc:
        nsrc += 1
    srcb = tot // nsrc
    assert srcb <= SRCB
    return nsrc, srcb


def _bcast_mid(ap, n):
    return bass.AP(ap.tensor, ap.offset, [ap.ap[0], [0, n], ap.ap[1]])


def _bcast_last(ap, n):
    return bass.AP(ap.tensor, ap.offset, [ap.ap[0], ap.ap[1], [0, n]])


def build_nc(VQ, meta):
    dt = mybir.dt
    NCH = VQ // CHROWS
    assert VQ % CHROWS == 0
    nsrc, srcb = _src_layout(VQ)
    caps = meta["caps"]          # [NCH][nsrc] int (mult of 128)
    pairs = meta["pairs"]        # [NCH][nsrc] list[(j, g)]
    ioff = meta["ioff"]          # [NCH][nsrc] idx col offset (16-wrapped)
    poff = meta["poff"]          # [NCH][nsrc] pair col offset
    toti = meta["toti"]
    totp = meta["totp"]

    nc = bacc.Bacc(None, num_devices=NCORES, debug=False, num_swdge_queues=4)

    xtab = nc.declare_dram_parameter("xtab", [NGROUP * VQ, FEAT], dt.bfloat16, isOutput=False)
    x0ownT = nc.declare_dram_parameter("x0ownT", [2, 128, VQ], dt.bfloat16, isOutput=False)
    gidx = nc.declare_dram_parameter("gidx", [128, toti], dt.int16, isOutput=False)
    gval = nc.declare_dram_parameter("gval", [128, totp], dt.bfloat16, isOutput=False)
    grow = nc.declare_dram_parameter("grow", [128, totp], dt.bfloat16, isOutput=False)
    iota128 = nc.declare_dram_parameter("iota128", [128, 128], dt.bfloat16, isOutput=False)
    wmat = nc.declare_dram_parameter("wmat", [KK, 128, 64], dt.bfloat16, isOutput=False)
    bias_rep = nc.declare_dram_parameter("bias_rep", [128, 64], dt.float32, isOutput=False)
    out_ext = nc.declare_dram_parameter("out", [4, VQ, 64], dt.float32, isOutput=True)

    y1 = nc.dram_tensor("y1tab", [VQ, FEAT], dt.bfloat16, kind="Internal")
    x1all = nc.dram_tensor("x1all", [NGROUP * VQ, FEAT], dt.bfloat16,
                           kind="Internal", addr_space="Shared")

    groups = [[0, 1, 2, 3, 4, 5, 6, 7]]

    with tile.TileContext(nc) as tc:
        with (
            tc.tile_pool(name="sb", bufs=5) as sb,
            tc.tile_pool(name="ysb", bufs=8) as ysbp,
            tc.tile_pool(name="xt", bufs=2) as xtp,
            tc.tile_pool(name="consts", bufs=1) as consts,
            tc.tile_pool(name="psum", bufs=8, space="PSUM") as pp,
        ):
            iota_t = consts.tile([128, 128], dt.bfloat16, tag="iota")
            nc.sync.dma_start(iota_t[:, :], iota128[:, :])
            w_ts = []
            for t in range(KK):
                w_t = consts.tile([128, 64], dt.bfloat16, tag=f"w{t}")
                nc.sync.dma_start(w_t[:, :], wmat[t, :, :])
                w_ts.append(w_t)
            bias_t = consts.tile([128, 64], dt.float32, tag="bias")
            nc.sync.dma_start(bias_t[:, :], bias_rep[:, :])
            ident_t = consts.tile([128, 128], dt.bfloat16, tag="ident")
            make_identity(nc, ident_t[:, :])
            tc.strict_bb_all_engine_barrier()

            qctr = [0]

            def spmm(src_table, ytab, epilogue=None):
                for c in range(NCH):
                    banks = []
                    for _bi in range(4):
                        bank_t = pp.tile([128, 512], dt.float32, tag="ps",
                                         bufs=6)
                        banks.append(bank_t)
                    # start/stop flags per BANK: matmul start zeroes the whole
                    # 2KB psum row on the addressed partitions, so only the
                    # bank's chronologically-first matmul may set start.
                    gfirst = {}
                    glast = {}
                    for s in range(nsrc):
                        for pi, (j, g) in enumerate(pairs[c][s]):
                            if g // 2 not in gfirst:
                                gfirst[g // 2] = (s, pi)
                            glast[g // 2] = (s, pi)
                    assert len(gfirst) == 4, f"chunk {c}: empty bank"
                    for s in range(nsrc):
                        cap = caps[c][s]
                        tcn = cap // 128
                        np_ = len(pairs[c][s])
                        idx_t = sb.tile([128, tcn * 8], dt.int16, tag="idx")
                        nc.sync.dma_start(
                            idx_t[:, :], gidx[:, ioff[c][s]:ioff[c][s] + tcn * 8])
                        row_t = sb.tile([128, np_], dt.bfloat16, tag="row")
                        nc.sync.dma_start(
                            row_t[:, :], grow[:, poff[c][s]:poff[c][s] + np_])
                        val_t = sb.tile([128, np_], dt.bfloat16, tag="val")
                        nc.sync.dma_start(
                            val_t[:, :], gval[:, poff[c][s]:poff[c][s] + np_])
                        g_t = sb.tile([128, tcn, FEAT], dt.bfloat16, tag="g",
                                      bufs=7)
                        # device limit: <=1024 idxs per dma_gather call
                        for q0 in range(0, cap, MAXCALL):
                            n = min(MAXCALL, cap - q0)
                            nc.gpsimd.dma_gather(
                                out_ap=g_t[:, q0 // 128:(q0 + n) // 128, :],
                                in_ap=src_table[s * srcb:(s + 1) * srcb, :],
                                idxs_ap=idx_t[:, q0 // 16:(q0 + n) // 16],
                                num_idxs=n, num_idxs_reg=n,
                                elem_size=FEAT,
                                queue_num=qctr[0] % 4,
                                single_packet=SINGLE_PACKET,
                            )
                            qctr[0] += 1
                        eq_t = sb.tile([128, np_, 128], dt.bfloat16, tag="eq",
                                       bufs=3)
                        nc.vector.tensor_tensor(
                            out=eq_t[:, :, :],
                            in0=_bcast_mid(iota_t[:, :], np_),
                            in1=_bcast_last(row_t[:, :], 128),
                            op=mybir.AluOpType.is_equal,
                        )
                        sel_t = sb.tile([128, np_, 128], dt.bfloat16, tag="sel")
                        nc.vector.tensor_tensor(
                            out=sel_t[:, :, :],
                            in0=eq_t[:, :, :],
                            in1=_bcast_last(val_t[:, :], 128),
                            op=mybir.AluOpType.mult,
                        )
                        for pi, (j, g) in enumerate(pairs[c][s]):
                            pt = banks[g // 2]
                            fo = (g % 2) * 256
                            nc.tensor.matmul(
                                pt[:, fo:fo + 256],
                                lhsT=sel_t[:, pi, :],
                                rhs=g_t[:, j, :],
                                start=(gfirst[g // 2] == (s, pi)),
                                stop=(glast[g // 2] == (s, pi)),
                                skip_group_check=True,
                            )
                    ysbs = []
                    for bk in range(4):
                        ysb = ysbp.tile([128, 512], dt.bfloat16, tag="ysb")
                        nc.scalar.copy(ysb[:, :], banks[bk][:, :])
                        ysbs.append(ysb)
                        if ytab is None:
                            continue
                        # dest row a*128 + p <- ysb[p, a*256:(a+1)*256]
                        dst = ytab[c * CHROWS + bk * 256:
                                   c * CHROWS + (bk + 1) * 256, :]
                        nc.sync.dma_start(
                            dst.rearrange("(a j) f -> j a f", a=2),
                            ysb[:, :].rearrange("p (a f) -> p a f", a=2))
                    if epilogue is not None:
                        epilogue(c, ysbs)

                tc.strict_bb_all_engine_barrier()

            def phase_d_chunk(c, ysbs):
                xts = []
                # x0 is host-pretransposed: plain DMA, no PE transposes
                x0t2 = []
                for half in range(2):
                    xt = xtp.tile([128, 8, 128], dt.bfloat16,
                                  tag=f"x0T{half}")
                    nc.sync.dma_start(
                        xt[:, :, :],
                        x0ownT[half, :, c * DROWS:(c + 1) * DROWS].rearrange(
                            "p (a c2) -> p a c2", a=8))
                    x0t2.append(xt)
                xts.append(x0t2)
                # y1 from its DRAM table
                xr = xtp.tile([128, 8, FEAT], dt.bfloat16, tag="y1Tr")
                nc.sync.dma_start(
                    xr[:, :, :],
                    y1[c * DROWS:(c + 1) * DROWS, :].rearrange(
                        "(a p) f -> p a f", p=128))
                xt2 = []
                for half in range(2):
                    xt = xtp.tile([128, 8, 128], dt.bfloat16,
                                  tag=f"y1T{half}")
                    for a8 in range(8):
                        ptt = pp.tile([128, 128], dt.bfloat16, tag="pst",
                                      bufs=2)
                        nc.tensor.transpose(
                            out=ptt[:, :],
                            in_=xr[:, a8, 128 * half:128 * (half + 1)],
                            identity=ident_t[:, :])
                        nc.scalar.copy(xt[:, a8, :], ptt[:, :])
                    xt2.append(xt)
                xts.append(xt2)
                # y2 straight from this chunk's psum copies in SBUF:
                # y2 row a8*128 + q <- ysbs[a8//2][q, (a8%2)*256 : ...]
                xt2 = []
                for half in range(2):
                    xt = xtp.tile([128, 8, 128], dt.bfloat16,
                                  tag=f"y2T{half}")
                    for a8 in range(8):
                        ptt = pp.tile([128, 128], dt.bfloat16, tag="pst",
                                      bufs=2)
                        fo = (a8 % 2) * 256 + 128 * half
                        nc.tensor.transpose(
                            out=ptt[:, :],
                            in_=ysbs[a8 // 2][:, fo:fo + 128],
                            identity=ident_t[:, :])
                        nc.scalar.copy(xt[:, a8, :], ptt[:, :])
                    xt2.append(xt)
                xts.append(xt2)
                for b in range(4):
                    pt = pp.tile([128, 512], dt.float32, tag="ps", bufs=6)
                    for j in range(8):
                        for t in range(KK):
                            nc.tensor.matmul(
                                pt[:, 64 * j:64 * (j + 1)],
                                lhsT=xts[t][b // 2][64 * (b % 2):64 * (b % 2 + 1), j, :],
                                rhs=w_ts[t][64 * (b % 2):64 * (b % 2 + 1), :],
                                start=(t == 0 and j == 0),
                                stop=(t == KK - 1 and j == 7),
                                skip_group_check=True,
                            )
                    osb = sb.tile([128, 8, 64], dt.float32, tag="osbD")
                    pt3 = bass.AP(pt[:, :].tensor, pt[:, :].offset,
                                  [pt[:, :].ap[0], [64, 8], [1, 64]])
                    nc.vector.tensor_tensor(
                        out=osb[:, :, :], in0=pt3,
                        in1=_bcast_mid(bias_t[:, :], 8),
                        op=mybir.AluOpType.add,
                    )
                    dst = out_ext[b, c * DROWS:(c + 1) * DROWS, :].rearrange(
                        "(j p) q -> p j q", p=128)
                    nc.sync.dma_start(dst, osb[:, :, :])

            ph = os.environ.get("KPHASES", "1234")
            PIECE = VQ // NSPLIT
            CPP = NCH // NSPLIT   # chunks per collective piece

            def issue_piece(p):
                nc.gpsimd.collective_compute(
                    "AllGather", mybir.AluOpType.bypass,
                    replica_groups=groups,
                    ins=[y1[p * PIECE:(p + 1) * PIECE, :]],
                    outs=[x1all[p * NGROUP * PIECE:(p + 1) * NGROUP * PIECE, :]],
                )

            def gather_piece(c, ysbs):
                # issue piece p one chunk after its last y1 chunk is written,
                # so the collective's y1-write wait never stalls the gathers
                if c % CPP == 0 and c > 0:
                    issue_piece(c // CPP - 1)

            if "1" in ph:
                spmm(xtab, y1,
                     epilogue=(gather_piece if "2" in ph else None))
                if "2" in ph:
                    issue_piece(NSPLIT - 1)

            if "3" in ph:
                spmm(x1all, None,
                     epilogue=(phase_d_chunk if "4" in ph else None))

    nc.finalize()
    return nc


# --------------------------------------------------------------------------
# host-side preparation
# --------------------------------------------------------------------------

def prepare_inputs(lap_vals, x, weight, bias, lap_rows, lap_cols):
    vq = V // NGROUP
    nch = vq // CHROWS
    nsrc, srcb = _src_layout(vq)

    rows = np.asarray(lap_rows).astype(np.int64)
    cols = np.asarray(lap_cols).astype(np.int64)
    vals = np.asarray(lap_vals).astype(np.float32)
    x = np.asarray(x)
    weight = np.asarray(weight)
    bias = np.asarray(bias)

    owner = rows % NGROUP
    rid = rows // NGROUP
    c_of = rid // CHROWS
    grp = (rid % CHROWS) // GR
    rlo = rid % GR
    # piece-major table layout so the AllGather can be issued per piece:
    # graph row v at table pos piece*(8*PIECE) + (v%8)*PIECE + (v//8)%PIECE
    piece_sz = vq // NSPLIT
    rid_c = cols // NGROUP
    tabpos = ((rid_c // piece_sz) * (NGROUP * piece_sz)
              + (cols % NGROUP) * piece_sz + rid_c % piece_sz)
    blk = tabpos // srcb
    cloc = (tabpos % srcb).astype(np.int16)

    # per-core sorted edge streams and per-cell/group counts
    ncell = nch * nsrc
    key_cell = c_of * nsrc + blk
    key_full = (key_cell * NGRP + grp)

    cnt_cell = np.zeros((NCORES, nch, nsrc), np.int64)
    cnt_grp = np.zeros((NCORES, nch, nsrc, NGRP), np.int64)
    order_of = []
    for h in range(NCORES):
        e_h = np.where(owner == h)[0]
        o = e_h[np.argsort(key_full[e_h], kind="stable")]
        order_of.append(o)
        np.add.at(cnt_cell, (h, c_of[o], blk[o]), 1)
        np.add.at(cnt_grp, (h, c_of[o], blk[o], grp[o]), 1)

    capgrain = int(os.environ.get("KCAPGRAIN", "128"))
    caps = (-(-cnt_cell.max(axis=0) // capgrain) * capgrain).astype(np.int64)

    # union pair list per cell: (j, g) if any core's group-g token range
    # intersects tile j
    gend = np.cumsum(cnt_grp, axis=3)          # [h, c, s, g] end positions
    gstart = gend - cnt_grp
    pairs = []
    for c in range(nch):
        prow = []
        for s in range(nsrc):
            pset = set()
            for h in range(NCORES):
                for g in range(NGRP):
                    a, b_ = gstart[h, c, s, g], gend[h, c, s, g]
                    if b_ > a:
                        for j in range(int(a) // 128, (int(b_) - 1) // 128 + 1):
                            pset.add((j, g))
            prow.append(sorted(pset))
        pairs.append(prow)

    ioff = np.zeros((nch, nsrc), np.int64)
    poff = np.zeros((nch, nsrc), np.int64)
    io = po = 0
    for c in range(nch):
        for s in range(nsrc):
            ioff[c][s] = io
            poff[c][s] = po
            io += caps[c][s] // 16
            po += len(pairs[c][s])
    toti, totp = io, po

    # per-core streams
    gidx_c, gval_c, grow_c = [], [], []
    for h in range(NCORES):
        o = order_of[h]
        # token position within cell = rank within (c, s) cell
        cellk = key_cell[o]
        cnt_flat = cnt_cell[h].reshape(-1)
        starts = np.concatenate([[0], np.cumsum(cnt_flat)[:-1]])
        # edges sorted by key_full -> within cell sorted by grp; position:
        pos_in_cell = np.arange(len(o)) - starts[cellk]

        gidx = np.zeros((128, toti), np.int16)
        gval = np.zeros((128, totp), BF16)
        growm = np.full((128, totp), 255.0, BF16)

        idx_buf = np.zeros(int(caps.sum()), np.int16)
        # flat per-cell idx array, zero padded
        cell_i0 = np.concatenate([[0], np.cumsum(caps.reshape(-1))[:-1]])
        idx_buf[cell_i0[cellk] + pos_in_cell] = cloc[o]
        # wrap 16: idx i -> [i%16, i//16], broadcast to 128 partitions
        for c in range(nch):
            for s in range(nsrc):
                cap = caps[c][s]
                ci = cell_i0[c * nsrc + s]
                w = idx_buf[ci:ci + cap].reshape(cap // 16, 16).T  # [16, cap/16]
                gidx[:, ioff[c][s]:ioff[c][s] + cap // 16] = np.tile(w, (8, 1))

        # pair streams: edges are (cell, grp)-sorted so each pair's tokens
        # are the contiguous positions [max(128j, gstart), min(128j+128, gend))
        val_o = vals[o].astype(BF16)
        rlo_o = rlo[o].astype(np.float32).astype(BF16)
        for c in range(nch):
            for s in range(nsrc):
                base = starts[c * nsrc + s]
                for pi, (j, g) in enumerate(pairs[c][s]):
                    col = poff[c][s] + pi
                    a = max(128 * j, int(gstart[h, c, s, g]))
                    b_ = min(128 * j + 128, int(gend[h, c, s, g]))
                    if a >= b_:
                        continue
                    pp_ = np.arange(a - 128 * j, b_ - 128 * j)
                    gval[pp_, col] = val_o[base + a:base + b_]
                    growm[pp_, col] = rlo_o[base + a:base + b_]
        gidx_c.append(gidx)
        gval_c.append(gval)
        grow_c.append(growm)

    # table (piece-major layout, must match tabpos above)
    feat = np.concatenate([x[0], x[1], x[2], x[3]], axis=1).astype(BF16)
    v_all = np.arange(V)
    rid_all = v_all // NGROUP
    tp_all = ((rid_all // piece_sz) * (NGROUP * piece_sz)
              + (v_all % NGROUP) * piece_sz + rid_all % piece_sz)
    xtab = np.zeros((NGROUP * vq, FEAT), BF16)
    xtab[tp_all] = feat
    # x0ownT: core h's rows in rid order, host-pretransposed to
    # [half, feat-in-half, row] so phase D needs no PE transposes for x0
    x0ownT_c = [np.ascontiguousarray(
        feat[np.arange(vq) * NGROUP + h].T.reshape(2, 128, vq))
        for h in range(NCORES)]

    iota128 = np.broadcast_to(np.arange(128, dtype=np.float32)[None, :],
                              (128, 128)).astype(BF16).copy()

    wm = weight.reshape(KK * P, Q)
    wk = wm.reshape(P, KK, Q).transpose(1, 0, 2)
    wfix = np.stack([wk[0] - wk[2], wk[1], 2.0 * wk[2]])
    wfix = np.concatenate([wfix, wfix], axis=1).astype(BF16)

    bias_rep = np.tile(np.asarray(bias, np.float32)[None, :], (128, 1))

    meta = {
        "caps": caps.tolist(), "pairs": pairs,
        "ioff": ioff.tolist(), "poff": poff.tolist(),
        "toti": int(toti), "totp": int(totp),
    }

    in_maps = []
    for h in range(NCORES):
        in_maps.append({
            "xtab": xtab,
            "x0ownT": x0ownT_c[h],
            "gidx": gidx_c[h], "gval": gval_c[h], "grow": grow_c[h],
            "iota128": iota128,
            "wmat": wfix, "bias_rep": bias_rep,
        })
    return in_maps, vq, meta


def assemble_output(results, vq, meta):
    out = np.zeros((B, V, Q), np.float32)
    for core in range(NCORES):
        co = results[core]["out"]
        v_ids = np.where(np.arange(V) % NGROUP == core)[0]
        for b in range(B):
            out[b, v_ids] = co[b]
    return out


def kernel(lap_vals, x, weight, bias, lap_rows, lap_cols):
    from concourse.bass_utils import run_bass_kernel_spmd

    in_maps, vq, meta = prepare_inputs(
        lap_vals, x, weight, bias, lap_rows, lap_cols)
    nc = build_nc(vq, meta)
    res = run_bass_kernel_spmd(nc, in_maps, core_ids=list(range(NCORES)))
    return assemble_output(res.results, vq, meta)
